# revision 5
# baseline (speedup 1.0000x reference)
"""MD-RNN (4-direction 2D GRU) Trainium2 kernel, v2: deep psum pipelining.

Sharding: 8-way data-parallel over batch (B=256 -> 32 per core); each core runs
all 4 directional 2D-GRU scans as anti-diagonal wavefronts, interleaved.

v2 changes vs baseline:
  - work unit = (chunk, kc): one 128-row hidden half of a <=16-cell chunk.
    psum per unit: P_rz (2 banks, r|z) from pool bufs=2 + P_n (1 bank) from
    pool bufs=3 -> 7/8 banks, 2-3 units in flight, PE never waits on EW.
  - t2 eliminated: DVE multiplies r into P_n *in place* (psum rmw), then the
    xn matmul accumulates on top (has_written bits stay set from uh_n/uh2_n),
    tanh reads psum directly.  Validated exact on HW (mini_psum_trick.py).
  - wx input projection: normal-mode K=17 fp8 matmuls (bias via ones row),
    FWL weight loads (~27ns) instead of DoubleRow's 213ns.
  - recurrence: DoubleRow fp8 for fd>=256, normal-mode (FWL) below.
  - skip path: htb stores full-scale h; dt = 0.5*st - n via
    scalar_tensor_tensor; the h' add writes htb directly; one fp8 cast.
"""

import os

import numpy as np
import ml_dtypes

SKIP_EW = int(os.environ.get("V2_SKIP_EW", "0"))   # timing exp: matmuls only
SKIP_MM = int(os.environ.get("V2_SKIP_MM", "0"))   # timing exp: EW only

GRID = 4
N_IMG = 32
S = N_IMG - (GRID - 1)          # 29 patch positions per axis
B_FULL = 256
N_CORES = 8
B = B_FULL // N_CORES           # 32 batch per core
H = 256
H3 = 3 * H                      # 768
OUT_DIM = 10
K_IN = GRID * GRID + 1          # 16 patch elems + ones row (bias trick)

FWD = list(range(S))                 # 29 entries
BWD = list(range(S - 2, -1, -1))     # 28 entries (reference off-by-one kept)
DIRS = [(FWD, FWD), (BWD, FWD), (FWD, BWD), (BWD, BWD)]

CELLS_PER_CHUNK = 16            # 16 cells * B=32 = 512 cols
CPB = CELLS_PER_CHUNK * B
DR_MIN_FD = int(os.environ.get("V2_DR_MIN_FD", "256"))  # below: normal-mode rec matmuls
REPEAT = 1                      # body repetitions (timing calibration only)

F8 = ml_dtypes.float8_e4m3
BF = ml_dtypes.bfloat16


def _diag_infos():
    """Per direction: list over diagonals of (i_lo, i_hi, global cell base)."""
    infos = []
    base = 0
    for (yi, xi) in DIRS:
        ny, nx = len(yi), len(xi)
        diags = []
        for d in range(ny + nx - 1):
            ilo = max(0, d - (nx - 1))
            ihi = min(d, ny - 1)
            diags.append((ilo, ihi, base))
            base += ihi - ilo + 1
        infos.append(diags)
    return infos, base


DIAG_INFOS, TOT_CELLS = _diag_infos()


def _scan_index_arrays():
    """Image-space (y, x) of every cell in pt order (dir-major, diag-major)."""
    ys, xs = [], []
    for a, (yi, xi) in enumerate(DIRS):
        for d, (ilo, ihi, _) in enumerate(DIAG_INFOS[a]):
            for i in range(ilo, ihi + 1):
                ys.append(yi[i])
                xs.append(xi[d - i])
    return np.asarray(ys), np.asarray(xs)


YS, XS = _scan_index_arrays()


def _chunk_sizes(k):
    nch = (k + CELLS_PER_CHUNK - 1) // CELLS_PER_CHUNK
    lo = k // nch
    rem = k - lo * nch
    return [lo + 1] * rem + [lo] * (nch - rem)


def make_pt(xc):
    """(B, 32, 32) core batch slice -> (17, TOT_CELLS*B) fp8 patch matrix.

    Rows 0..15 = flattened 4x4 patch (row-major); row 16 = ones (bias row).
    """
    from numpy.lib.stride_tricks import sliding_window_view
    w = sliding_window_view(xc, (GRID, GRID), axis=(1, 2))   # (B, 29, 29, 4, 4)
    p = w[:, YS, XS].reshape(xc.shape[0], TOT_CELLS, GRID * GRID)  # (B, T, 16)
    p = np.ascontiguousarray(p.transpose(2, 1, 0)).reshape(GRID * GRID, -1)
    pt = np.empty((K_IN, p.shape[1]), np.float32)
    pt[:GRID * GRID] = p
    pt[GRID * GRID] = 1.0
    return np.ascontiguousarray(pt.astype(F8))


def make_weight_maps(Wx, Uh, Uh2, b, W_out, b_out):
    Wx, Uh, Uh2 = (np.asarray(t, np.float32) for t in (Wx, Uh, Uh2))
    b, W_out, b_out = (np.asarray(t, np.float32) for t in (b, W_out, b_out))
    # recurrence weights as lhsT [a, Ki=128, Ko(kt)=2, 3H]; works for both
    # DoubleRow ([:, :, csl]) and normal ([:, kt, csl]) access.
    uh = np.empty((4, 128, 2, H3), np.float32)
    uh2 = np.empty((4, 128, 2, H3), np.float32)
    for a in range(4):
        for kt in range(2):
            uh[a, :, kt, :] = Uh[a][kt * 128:(kt + 1) * 128]
            uh2[a, :, kt, :] = Uh2[a][kt * 128:(kt + 1) * 128]
    # input projection [a, 17, kc, 3*128]: col g*128+c = Wx col g*256+kc*128+c
    wx = np.empty((4, K_IN, 2, 384), np.float32)
    for a in range(4):
        for kc in range(2):
            for g in range(3):
                src = slice(g * 256 + kc * 128, g * 256 + kc * 128 + 128)
                wx[a, :GRID * GRID, kc, g * 128:(g + 1) * 128] = Wx[a][:, src]
                wx[a, GRID * GRID, kc, g * 128:(g + 1) * 128] = b[a][src]
    wo = np.ascontiguousarray(W_out.reshape(8, 128, OUT_DIM).astype(BF))
    bo = np.ascontiguousarray(b_out.reshape(1, OUT_DIM))
    return {
        "uh": np.ascontiguousarray(uh.astype(F8)),
        "uh2": np.ascontiguousarray(uh2.astype(F8)),
        "wx": np.ascontiguousarray(wx.astype(F8)),
        "wo": wo,
        "bo": bo,
    }


def _build_nc():
    import concourse.bacc as bacc
    import concourse.mybir as mybir
    import concourse.tile as tile

    f32 = mybir.dt.float32
    f8 = mybir.dt.float8e4
    bf16 = mybir.dt.bfloat16
    AF = mybir.ActivationFunctionType
    ALU = mybir.AluOpType
    DR = mybir.MatmulPerfMode.DoubleRow

    nc = bacc.Bacc("TRN2", target_bir_lowering=False, debug=False,
                   num_devices=N_CORES)
    pt_d = nc.dram_tensor("pt", [K_IN, TOT_CELLS * B], f8, kind="ExternalInput")
    uh_d = nc.dram_tensor("uh", [4, 128, 2, H3], f8, kind="ExternalInput")
    uh2_d = nc.dram_tensor("uh2", [4, 128, 2, H3], f8, kind="ExternalInput")
    wx_d = nc.dram_tensor("wx", [4, K_IN, 2, 384], f8, kind="ExternalInput")
    wo_d = nc.dram_tensor("wo", [8, 128, OUT_DIM], bf16, kind="ExternalInput")
    bo_d = nc.dram_tensor("bo", [1, OUT_DIM], f32, kind="ExternalInput")
    out_d = nc.dram_tensor("out", [B, OUT_DIM], f32, kind="ExternalOutput")

    with tile.TileContext(nc) as tc:
        from contextlib import ExitStack
        with ExitStack() as ctx:
            const = ctx.enter_context(tc.tile_pool(name="const", bufs=1))
            ptp = ctx.enter_context(tc.tile_pool(name="ptp", bufs=8))
            prz = ctx.enter_context(tc.tile_pool(name="prz", bufs=2,
                                                 space="PSUM"))
            pn = ctx.enter_context(tc.tile_pool(name="pn", bufs=4,
                                                space="PSUM"))
            hps = [ctx.enter_context(tc.tile_pool(name=f"h{a}", bufs=3))
                   for a in range(4)]
            ew = ctx.enter_context(tc.tile_pool(name="ew", bufs=6))
            hd = ctx.enter_context(tc.tile_pool(name="hd", bufs=1))

            # --- resident weights ---
            uh_sb, uh2_sb, wx_sb = {}, {}, {}
            for a in range(4):
                t = const.tile([128, 2, H3], f8, tag=f"uh{a}")
                nc.sync.dma_start(out=t, in_=uh_d[a])
                uh_sb[a] = t
                t = const.tile([128, 2, H3], f8, tag=f"uh2{a}")
                nc.sync.dma_start(out=t, in_=uh2_d[a])
                uh2_sb[a] = t
                t = const.tile([K_IN, 2, 384], f8, tag=f"wx{a}")
                nc.sync.dma_start(out=t, in_=wx_d[a])
                wx_sb[a] = t
            wo_sb = const.tile([128, 8 * OUT_DIM], bf16, tag="wo")
            for c in range(8):
                nc.sync.dma_start(out=wo_sb[:, c * OUT_DIM:(c + 1) * OUT_DIM],
                                  in_=wo_d[c])
            bo_sb = const.tile([1, OUT_DIM], f32, tag="bo")
            nc.sync.dma_start(out=bo_sb, in_=bo_d[:, :])
            ones_sb = const.tile([1, B], f32, tag="ones")
            nc.vector.memset(ones_sb, 1.0)
            zero_h8 = const.tile([128, 2, 2 * B], f8, tag="zeroh8")
            nc.vector.memset(zero_h8, 0.0)
            zero_hb = const.tile([128, 2, 2 * B], bf16, tag="zerohb")
            nc.vector.memset(zero_hb, 0.0)

            ucnt = [0]

            def emit_unit(a, kc, prev8, prevb, s_a, c0, c1, ht8, htb, ptt):
                ucnt[0] += 1
                """One (chunk, kc) unit: 9ish matmuls + 2 act + 5 DVE/GP ops."""
                fd = (c1 - c0) * B
                o_a = (s_a + c0) * B
                o_l = (s_a + 1 + c0) * B
                above8 = prev8[:, :, o_a:o_a + fd]
                left8 = prev8[:, :, o_l:o_l + fd]
                above_b = prevb[:, kc, o_a:o_a + fd]
                left_b = prevb[:, kc, o_l:o_l + fd]

                P_rz = prz.tile([128, 2, CPB], f32, tag="prz")
                P_n = pn.tile([128, CPB], f32, tag="pn")
                dr = fd >= DR_MIN_FD

                def rec_mms(po, csl, first_start, last_stop):
                    if SKIP_MM:
                        return
                    if dr:
                        nc.tensor.matmul(po, uh_sb[a][:, :, csl], above8,
                                         perf_mode=DR, start=first_start,
                                         stop=False, skip_group_check=True)
                        nc.tensor.matmul(po, uh2_sb[a][:, :, csl], left8,
                                         perf_mode=DR, start=False,
                                         stop=last_stop, skip_group_check=True)
                    else:
                        for wi, (w_sb, rhs) in enumerate(
                                ((uh_sb[a], above8), (uh2_sb[a], left8))):
                            for kt in (0, 1):
                                nc.tensor.matmul(
                                    po, w_sb[:, kt, csl], rhs[:, kt, :],
                                    start=first_start and wi == 0 and kt == 0,
                                    stop=last_stop and wi == 1 and kt == 1,
                                    skip_group_check=True)

                # r and z gates -> P_rz banks (wx opens each bank's group)
                for g in (0, 1):
                    po = P_rz[:, g, :fd]
                    csl = slice(g * 256 + kc * 128, g * 256 + kc * 128 + 128)
                    if not SKIP_MM:
                        nc.tensor.matmul(po,
                                         wx_sb[a][:, kc, g * 128:(g + 1) * 128],
                                         ptt[:, :fd], start=True, stop=False,
                                         skip_group_check=True)
                    rec_mms(po, csl, False, True)
                # n gate recurrence opens P_n's group; xn lands after t1
                csl_n = slice(512 + kc * 128, 512 + kc * 128 + 128)
                rec_mms(P_n[:, :fd], csl_n, True, False)

                if SKIP_EW:
                    # keep the xn matmul in the PE stream, skip all EW
                    if not SKIP_MM:
                        nc.tensor.matmul(P_n[:, :fd],
                                         wx_sb[a][:, kc, 256:384],
                                         ptt[:, :fd], start=False, stop=True,
                                         skip_group_check=True)
                    return
                # sigmoid r|z (psum -> bf16 sbuf)
                rzt = ew.tile([128, 2, CPB], bf16, tag="rzt")
                nc.scalar.activation(rzt[:, :, :fd], P_rz[:, :, :fd],
                                     AF.Sigmoid)
                # t1: P_n *= r in place (psum rmw; has_written bits stay set)
                nc.vector.tensor_mul(P_n[:, :fd], P_n[:, :fd], rzt[:, 0, :fd])
                # xn accumulates on top of r*(hn+h2n)
                if not SKIP_MM:
                    nc.tensor.matmul(P_n[:, :fd], wx_sb[a][:, kc, 256:384],
                                     ptt[:, :fd], start=False, stop=True,
                                     skip_group_check=True)
                # n = tanh(psum) directly
                nt = ew.tile([128, CPB], bf16, tag="nt")
                nc.scalar.activation(nt[:, :fd], P_n[:, :fd], AF.Tanh)

                # skip path: st = h_above + h_left (full scale), on GpSimd
                st = ew.tile([128, CPB], bf16, tag="st")
                nc.gpsimd.tensor_add(st[:, :fd], above_b, left_b)
                # dt = 0.5*st - n, split so both ops run at DVE 4x/2x
                # modes instead of the 1x scalar_tensor_tensor
                sh = ew.tile([128, CPB], bf16, tag="sh")
                nc.vector.tensor_scalar_mul(sh[:, :fd], st[:, :fd], 0.5)
                dt = ew.tile([128, CPB], bf16, tag="dt")
                nc.vector.tensor_tensor(dt[:, :fd], sh[:, :fd], nt[:, :fd],
                                        ALU.subtract)
                # et = z * dt
                et = ew.tile([128, CPB], bf16, tag="et")
                nc.vector.tensor_mul(et[:, :fd], rzt[:, 1, :fd], dt[:, :fd])
                # h' = et + n -> htb directly (bf16, full scale)
                ob = htb[:, kc, (1 + c0) * B:(1 + c0) * B + fd]
                nc.vector.tensor_add(ob, et[:, :fd], nt[:, :fd])
                # fp8 copy for the next matmuls; alternate DVE/Act to
                # relieve DVE (measured EW-only stream 1.39ms > PE 1.22ms)
                o8 = ht8[:, kc, (1 + c0) * B:(1 + c0) * B + fd]
                if ucnt[0] % 2:
                    nc.scalar.copy(o8, ob)
                else:
                    nc.vector.tensor_copy(o8, ob)

            # --- main wavefront, 4 directions interleaved per diagonal ---
            max_nd = max(len(di) for di in DIAG_INFOS)
            for _rep in range(REPEAT):
              h_prev = {a: None for a in range(4)}
              for d in range(max_nd):
                 for a in range(4):
                    if d >= len(DIAG_INFOS[a]):
                        continue
                    ilo, ihi, cbase = DIAG_INFOS[a][d]
                    k = ihi - ilo + 1
                    # pad usage by the NEXT diagonal: left pad (col 0) only
                    # read when its span still starts at this ilo; right pad
                    # (col k+1) read when its span reaches past col k.
                    if d + 1 < len(DIAG_INFOS[a]):
                        nlo, nhi, _ = DIAG_INFOS[a][d + 1]
                        sa_n = nlo - ilo
                        need_l = sa_n == 0
                        need_r = sa_n + (nhi - nlo + 1) >= k
                    else:
                        need_l = need_r = False
                    ht8 = hps[a].tile([128, 2, (k + 2) * B], f8, tag=f"h{a}")
                    htb = hps[a].tile([128, 2, (k + 2) * B], bf16, tag=f"hb{a}")
                    if SKIP_EW:
                        nc.vector.memset(ht8, 0.25)
                        nc.gpsimd.memset(htb, 0.125)
                    for t in (ht8, htb):
                        if need_l:
                            nc.vector.memset(t[:, :, 0:B], 0.0)
                        if need_r:
                            nc.gpsimd.memset(t[:, :, (k + 1) * B:(k + 2) * B],
                                             0.0)
                    if d == 0:
                        prev8, prevb, k_prev, ilo_prev = zero_h8, zero_hb, 0, 0
                    else:
                        prev8, prevb, k_prev, ilo_prev = h_prev[a]
                    s_a = ilo - ilo_prev
                    assert 0 <= s_a and s_a + k <= k_prev + 2, (a, d)
                    c0 = 0
                    for cs in _chunk_sizes(k):
                        ptt = ptp.tile([K_IN, CPB], f8, tag="pt")
                        nc.sync.dma_start(
                            out=ptt[:, :cs * B],
                            in_=pt_d[:, (cbase + c0) * B:(cbase + c0 + cs) * B])
                        for kc in (0, 1):
                            emit_unit(a, kc, prev8, prevb, s_a, c0, c0 + cs,
                                      ht8, htb, ptt)
                        c0 += cs
                    h_prev[a] = (ht8, htb, k, ilo)

            # --- head: logits = hcat @ W_out + b_out ; log_softmax ---
            pl_t = prz.tile([128, 2, CPB], f32, tag="prz")
            pl = pl_t[:B, 0, :OUT_DIM]
            for c in range(8):
                a, kc = divmod(c, 2)
                _, htb, k, _ = h_prev[a]
                assert k == 1
                nc.tensor.matmul(pl, htb[:, kc, B:2 * B],
                                 wo_sb[:, c * OUT_DIM:(c + 1) * OUT_DIM],
                                 start=(c == 0), stop=False,
                                 skip_group_check=True)
            nc.tensor.matmul(pl, ones_sb[:1, :B], bo_sb, start=False,
                             stop=True, skip_group_check=True)
            mx = hd.tile([B, 1], f32, tag="mx")
            nc.vector.reduce_max(mx, pl, axis=mybir.AxisListType.X)
            nmx = hd.tile([B, 1], f32, tag="nmx")
            nc.vector.tensor_scalar_mul(nmx, mx, -1.0)
            exv = hd.tile([B, OUT_DIM], f32, tag="exv")
            nc.scalar.activation(exv, pl, AF.Exp, bias=nmx, scale=1.0)
            sm = hd.tile([B, 1], f32, tag="sm")
            nc.vector.reduce_sum(sm, exv, axis=mybir.AxisListType.X)
            lnz = hd.tile([B, 1], f32, tag="lnz")
            nc.scalar.activation(lnz, sm, AF.Ln)
            tot = hd.tile([B, 1], f32, tag="tot")
            nc.vector.tensor_add(tot, lnz, mx)
            ntot = hd.tile([B, 1], f32, tag="ntot")
            nc.vector.tensor_scalar_mul(ntot, tot, -1.0)
            ot = hd.tile([B, OUT_DIM], f32, tag="ot")
            nc.scalar.activation(ot, pl, AF.Identity, bias=ntot, scale=1.0)
            nc.sync.dma_start(out=out_d[:, :], in_=ot)

    nc.compile()
    return nc


_CACHE = {}


def get_nc():
    if "nc" not in _CACHE:
        _CACHE["nc"] = _build_nc()
    return _CACHE["nc"]


def make_in_maps(x, Wx, Uh, Uh2, b, W_out, b_out):
    x = np.asarray(x, np.float32)
    wm = make_weight_maps(Wx, Uh, Uh2, b, W_out, b_out)
    in_maps = []
    for c in range(N_CORES):
        xc = x[c * B:(c + 1) * B]
        m = dict(wm)
        m["pt"] = make_pt(xc)
        in_maps.append(m)
    return in_maps


def kernel(x, Wx, Uh, Uh2, b, W_out, b_out):
    from concourse.bass_utils import run_bass_kernel_spmd
    nc = get_nc()
    in_maps = make_in_maps(x, Wx, Uh, Uh2, b, W_out, b_out)
    res = run_bass_kernel_spmd(nc, in_maps, list(range(N_CORES)))
    out = np.concatenate([res.results[c]["out"] for c in range(N_CORES)], axis=0)
    return out.astype(np.float32)
